# revision 34
# baseline (speedup 1.0000x reference)
"""
Trainium2 Bass kernel for nn_DenseFeatureNumericEmbedding.

Computes, per feature f (F=128 independent tiny MLPs):
    h[b,f,:]   = relu(x[b,f] * w1[f,:] + b1[f,:])            # [B, F, H]
    out[b,f,:] = h[b,f,:] @ w2[f,:,:] + b2[f,:]              # [B, F, E]
    returns out.reshape(B, F*E)                              # [16384, 4096] fp32

Sharding: data-parallel over batch across 8 NeuronCores (2048 rows/core),
params replicated. No collectives; host transposes + concatenates shards.

Per-core dataflow (per 512-batch chunk, per quad of 4 features):
  L1   TensorE: 4 row-tiled K=2 matmuls (stationary [w1[f]; b1[f]],
       moving [xT[f]; ones]) -> pre [H=128, 512] per feature in PSUM.
  RELU ScalarE activation(Relu) on features 0,1; VectorE
       tensor_scalar_max(0) on features 2,3. PSUM fp32 -> SBUF bf16 hT.
  L2   TensorE: 4 col-tiled K=128 matmuls (stationary w2[f] [H,E=32]),
       4 features packed into one PSUM bank -> poutT [FE=128, 512].
  OUT  +b2 and cast to bf16, PSUM -> SBUF, on ScalarE (Identity+bias)
       or VectorE (tensor_scalar add) per a balance pattern.
  DMA  store outT [FE, BL] bf16 to DRAM (1KB contiguous runs).
Host: transpose [FE, BL] -> [BL, FE] per shard, upcast to fp32, concat.
No PE transposes, no output-stage copies: ACT/DVE do only the two
mandatory PSUM crossings (relu 33.5M + out 8.4M elems per core).
"""

import sys

sys.path.insert(0, "/opt/trn_rl_repo")

import numpy as np
import ml_dtypes

import concourse.bass as bass
import concourse.tile as tile
from concourse import bacc, mybir
from concourse.bass_utils import run_bass_kernel_spmd

BF16 = ml_dtypes.bfloat16

B = 16384
F = 128
H = 128
E = 32
NCORES = 8
BL = B // NCORES          # 2048 rows per core
CHUNK = 512               # batch columns per inner tile (1 PSUM bank fp32)
NCHUNK = BL // CHUNK      # 4
NQUAD = F // 4            # 32 quads of 4 features

CONFIG = {
    # out-pass engine per quad index (cycled): 'A' ScalarE, 'D' VectorE
    "OUT_PATTERN": "ADAAD",
    # relu engine for pre_a/pre_b: 'AD' = ACT does features 0,1; DVE 2,3
    "RELU_PATTERN": "AD",
    "LOOKAHEAD": 1,       # quads of L1 prefetch ahead of the relu/L2 stage
    "VARIANT_ID": 7,      # busts the NEFF cache between variants
}

_COMPILED = None


def _build_bass():
    nc = bacc.Bacc("TRN2", target_bir_lowering=False, debug=False,
                   num_devices=NCORES)
    dt = mybir.dt

    xt2 = nc.dram_tensor("xt2", [2 * F, BL], dt.bfloat16,
                         kind="ExternalInput").ap()
    # w1b1s rows: 2j + r  (j feature-in-quad, r 0=w1 / 1=b1), cols q*H + h
    w1b1s = nc.dram_tensor("w1b1s", [8, F * H], dt.bfloat16,
                           kind="ExternalInput").ap()
    w2s = nc.dram_tensor("w2s", [H, F * E], dt.bfloat16,
                         kind="ExternalInput").ap()
    b2qs = nc.dram_tensor("b2qs", [128, NQUAD], dt.float32,
                          kind="ExternalInput").ap()
    outT = nc.dram_tensor("outT", [F * E, BL], dt.bfloat16,
                          kind="ExternalOutput").ap()

    # xt2 rows: 8q + 2j + r  (q quad, j feature-in-quad, r 0=x / 1=ones)
    xt2_r = xt2.rearrange("(q j r) n -> r j q n", j=4, r=2)  # [2,4,NQUAD,BL]

    for _ in range(CONFIG["VARIANT_ID"]):
        nc.sync.nop()

    out_pat = CONFIG["OUT_PATTERN"]
    relu_pat = CONFIG["RELU_PATTERN"]

    with tile.TileContext(nc) as tc:
        with (
            tc.tile_pool(name="params", bufs=1) as params,
            tc.tile_pool(name="xq", bufs=2) as xq_pool,
            tc.tile_pool(name="h", bufs=4) as h_pool,
            tc.tile_pool(name="outq", bufs=4) as outq_pool,
            tc.tile_pool(name="pre", bufs=3, space="PSUM") as pre_pool,
            tc.tile_pool(name="pout", bufs=2, space="PSUM") as pout_pool,
        ):
            # Startup: spread loads over engine queues so their fixed DMA
            # latencies overlap instead of serializing on one ring.
            w1b1q_sb = params.tile([128, F * H], dt.bfloat16, tag="w1b1q")
            b2_sb = params.tile([128, NQUAD], dt.float32, tag="b2qs")
            w2_sb = params.tile([H, F * E], dt.bfloat16, tag="w2s")
            warm = params.tile([128, 2], dt.float32, tag="warm")

            def load_params():
                nc.sync.dma_start(out=b2_sb[:], in_=b2qs[:])
                # split by r: each DMA hits partitions {0,32,64,96} -> 4
                # SDMA engines instead of 1
                w1b1_v = w1b1q_sb[:].rearrange("(j u) m -> u j m", u=32)
                w1b1s_v = w1b1s.rearrange("(j r) m -> r j m", r=2)
                for r in range(2):
                    nc.sync.dma_start(out=w1b1_v[r], in_=w1b1s_v[r])
                nc.gpsimd.dma_start(out=w2_sb[:], in_=w2s[:])
                # Pre-warm ACT function tables during the startup DMA wait.
                nc.scalar.activation(warm[:, 0:1], b2_sb[:, 0:1],
                                     mybir.ActivationFunctionType.Relu)
                nc.scalar.activation(warm[:, 1:2], b2_sb[:, 0:1],
                                     mybir.ActivationFunctionType.Identity)

            NIT = NCHUNK * NQUAD
            LOOK = CONFIG["LOOKAHEAD"]
            xq_tiles = {}
            pre_tiles = {}

            def load_xq(c, split_first=False):
                # xq[32j + r, 512q + cc] = xt2[8q + 2j + r, 512c + cc]
                # DMAs split by r: each spans partitions {r,32+r,64+r,96+r}
                # so the transfer spreads over SDMA engines, on the scalar
                # ring (keeps the sync ring free for out-stores). For the
                # first chunk, a small priority DMA covers quads 0-7 so the
                # pipeline can start before the bulk transfer lands.
                xq = xq_pool.tile([128, NQUAD * CHUNK], dt.bfloat16, tag="xq")
                xq_v = xq[:].rearrange("(j u) (q n) -> u j q n",
                                       u=32, n=CHUNK)
                qsplits = [(0, 8), (8, NQUAD - 8)] if split_first \
                    else [(0, NQUAD)]
                for q0, qn in qsplits:
                    for r in range(2):
                        nc.scalar.dma_start(
                            out=xq_v[r, :, q0:q0 + qn, :],
                            in_=xt2_r[r, :, q0:q0 + qn, bass.ts(c, CHUNK)],
                        )
                xq_tiles[c] = xq

            def issue_l1(it):
                # ---- L1: 4 features, row-tiled, K=2 matmuls ----
                c, q = divmod(it, NQUAD)
                xq = xq_tiles[c]
                pre_a = pre_pool.tile([128, 2 * CHUNK], dt.float32, tag="pre")
                pre_b = pre_pool.tile([128, 2 * CHUNK], dt.float32, tag="pre")
                for j in range(4):
                    tgt = pre_a if j < 2 else pre_b
                    nc.tensor.matmul(
                        tgt[:, bass.ts(j % 2, CHUNK)],
                        lhsT=w1b1q_sb[32 * j:32 * j + 2, bass.ts(q, H)],
                        rhs=xq[32 * j:32 * j + 2, bass.ts(q, CHUNK)],
                        start=True, stop=True,
                        tile_position=(32 * j, 0),
                    )
                pre_tiles[it] = (pre_a, pre_b)

            def flush_out(pend):
                pout, it = pend
                q = it % NQUAD
                outq = outq_pool.tile([128, CHUNK], dt.bfloat16, tag="outq")
                eng = out_pat[it % len(out_pat)]
                if eng == "A":
                    nc.scalar.activation(
                        outq[:], pout[:],
                        mybir.ActivationFunctionType.Identity,
                        bias=b2_sb[:, q:q + 1],
                    )
                else:
                    nc.vector.tensor_scalar_add(
                        outq[:], pout[:], b2_sb[:, q:q + 1])
                c = it // NQUAD
                # alternate HWDGE rings: one ring's descriptor generation
                # (128 descs/store) can't keep up with the quad period
                ring = nc.sync if it % 2 == 0 else nc.scalar
                ring.dma_start(
                    out=outT[bass.ts(q, 128), bass.ts(c, CHUNK)],
                    in_=outq[:],
                )

            pending = None   # (pout_tile, it_idx) awaiting +b2/store
            load_xq(0, split_first=True)  # first so nothing delays it
            load_params()
            for it in range(LOOK):
                issue_l1(it)

            for it in range(NIT):
                c, q = divmod(it, NQUAD)
                # prefetch next chunk's x mid-way through this chunk
                if q == 8 and c + 1 < NCHUNK:
                    load_xq(c + 1)
                # L1 runs LOOK quads ahead of the relu/L2 stage
                if it + LOOK < NIT:
                    issue_l1(it + LOOK)

                pre_a, pre_b = pre_tiles.pop(it)
                # ---- relu + cast bf16, split ACT / DVE ----
                hT = h_pool.tile([128, 4 * CHUNK], dt.bfloat16, tag="h")
                for half, hsrc in ((0, pre_a), (1, pre_b)):
                    dst = hT[:, bass.ts(half, 2 * CHUNK)]
                    if relu_pat[half % len(relu_pat)] == "A":
                        nc.scalar.activation(
                            dst, hsrc[:],
                            mybir.ActivationFunctionType.Relu)
                    else:
                        nc.vector.tensor_scalar_max(dst, hsrc[:], 0.0)

                # ---- L2: 4 features col-tiled into one PSUM bank ----
                pout = pout_pool.tile([128, CHUNK], dt.float32, tag="pout")
                for j in range(4):
                    f = 4 * q + j
                    nc.tensor.matmul(
                        pout[32 * j:32 * j + 32, :],
                        lhsT=w2_sb[:, bass.ts(f, E)],
                        rhs=hT[:, bass.ts(j, CHUNK)],
                        start=True, stop=True,
                        tile_position=(0, 32 * j),
                    )

                # ---- previous quad's +b2 / cast / store ----
                if pending is not None:
                    flush_out(pending)
                pending = (pout, it)

            flush_out(pending)

    nc.compile()
    return nc


def _prep_inputs(x, w1, b1, w2, b2):
    """Host-side packing of parameters + per-core x shards."""
    w1b1s = np.zeros((8, F * H), dtype=BF16)
    for f in range(F):
        q, j = divmod(f, 4)
        w1b1s[2 * j + 0, H * q:H * q + H] = w1[f].astype(BF16)
        w1b1s[2 * j + 1, H * q:H * q + H] = b1[f].astype(BF16)

    w2s = np.ascontiguousarray(
        w2.transpose(1, 0, 2).reshape(H, F * E)).astype(BF16)
    # b2qs[32j + e, q] = b2[4q + j, e]
    b2qs = np.ascontiguousarray(
        b2.reshape(NQUAD, 4, E).transpose(1, 2, 0).reshape(128, NQUAD)
    ).astype(np.float32)

    in_maps = []
    for core in range(NCORES):
        xs = x[core * BL:(core + 1) * BL]          # [BL, F]
        xt2 = np.empty((2 * F, BL), dtype=BF16)
        xt2[0::2] = xs.T.astype(BF16)
        xt2[1::2] = BF16(1.0)
        in_maps.append({
            "xt2": xt2, "w1b1s": w1b1s, "w2s": w2s, "b2qs": b2qs,
        })
    return in_maps


def _get_compiled():
    global _COMPILED
    if _COMPILED is None:
        _COMPILED = _build_bass()
    return _COMPILED


def reset_compiled():
    global _COMPILED
    _COMPILED = None


def kernel(x, w1, b1, w2, b2, _trace=False, _trace_kwargs=None):
    nc = _get_compiled()
    in_maps = _prep_inputs(
        np.asarray(x, dtype=np.float32), np.asarray(w1, dtype=np.float32),
        np.asarray(b1, dtype=np.float32), np.asarray(w2, dtype=np.float32),
        np.asarray(b2, dtype=np.float32))
    res = run_bass_kernel_spmd(
        nc, in_maps, core_ids=list(range(NCORES)),
        trace=_trace, **(_trace_kwargs or {}))
    full = np.empty((B, F * E), dtype=np.float32)
    for i in range(NCORES):
        shard = np.asarray(res.results[i]["outT"])   # [FE, BL] bf16
        full[i * BL:(i + 1) * BL] = shard.T.astype(np.float32)
    if _trace:
        return full, res
    return full


if __name__ == "__main__":
    rng = np.random.default_rng(0)
    x = rng.standard_normal((B, F), dtype=np.float32)
    w1 = rng.standard_normal((F, H), dtype=np.float32)
    b1 = rng.standard_normal((F, H), dtype=np.float32)
    w2 = (rng.standard_normal((F, H, E), dtype=np.float32) / np.sqrt(H)).astype(np.float32)
    b2 = rng.standard_normal((F, E), dtype=np.float32) / np.sqrt(H)
    got = kernel(x=x, w1=w1, b1=b1, w2=w2, b2=b2)
    h = np.maximum(x[:, :, None] * w1[None] + b1[None], 0.0)
    want = (np.einsum("bfh,fhe->bfe", h, w2) + b2[None]).reshape(B, F * E)
    err = np.abs(got - want).max() / np.abs(want).max()
    print("self-test scale-relative max err:", err)


# revision 36
# speedup vs baseline: 1.1811x; 1.1811x over previous
"""
Trainium2 Bass kernel for nn_DenseFeatureNumericEmbedding.

Computes, per feature f (F=128 independent tiny MLPs):
    h[b,f,:]   = relu(x[b,f] * w1[f,:] + b1[f,:])            # [B, F, H]
    out[b,f,:] = h[b,f,:] @ w2[f,:,:] + b2[f,:]              # [B, F, E]
    returns out.reshape(B, F*E)                              # [16384, 4096] fp32

Sharding: data-parallel over batch across 8 NeuronCores (2048 rows/core),
params replicated. No collectives; host transposes + concatenates shards.

Per-core dataflow (per 512-batch chunk, per quad of 4 features):
  L1   TensorE: 4 row-tiled K=2 matmuls (stationary [w1[f]; b1[f]],
       moving [xT[f]; ones]) -> pre [H=128, 512] per feature in PSUM.
  RELU ScalarE activation(Relu) on features 0,1; VectorE
       tensor_scalar_max(0) on features 2,3. PSUM fp32 -> SBUF bf16 hT.
  L2   TensorE: 4 col-tiled K=128 matmuls (stationary w2[f] [H,E=32]),
       4 features packed into one PSUM bank -> poutT [FE=128, 512].
  OUT  +b2 and cast to bf16, PSUM -> SBUF, on ScalarE (Identity+bias)
       or VectorE (tensor_scalar add) per a balance pattern.
  DMA  store outT [FE, BL] bf16 to DRAM (1KB contiguous runs).
Host: transpose [FE, BL] -> [BL, FE] per shard, upcast to fp32, concat.
No PE transposes, no output-stage copies: ACT/DVE do only the two
mandatory PSUM crossings (relu 33.5M + out 8.4M elems per core).
"""

import sys

sys.path.insert(0, "/opt/trn_rl_repo")

import numpy as np
import ml_dtypes

import concourse.bass as bass
import concourse.tile as tile
from concourse import bacc, mybir
from concourse.bass_utils import run_bass_kernel_spmd

BF16 = ml_dtypes.bfloat16

B = 16384
F = 128
H = 128
E = 32
NCORES = 8
BL = B // NCORES          # 2048 rows per core
CHUNK = 512               # batch columns per inner tile (1 PSUM bank fp32)
NCHUNK = BL // CHUNK      # 4
NQUAD = F // 4            # 32 quads of 4 features

CONFIG = {
    # out-pass engine per quad index (cycled): 'A' ScalarE, 'D' VectorE
    "OUT_PATTERN": "ADAAD",
    # relu engine for pre_a/pre_b: 'AD' = ACT does features 0,1; DVE 2,3
    "RELU_PATTERN": "AD",
    "LOOKAHEAD": 1,       # quads of L1 prefetch ahead of the relu/L2 stage
    "VARIANT_ID": 8,      # busts the NEFF cache between variants
}

_COMPILED = None


def _build_bass():
    nc = bacc.Bacc("TRN2", target_bir_lowering=False, debug=False,
                   num_devices=NCORES)
    dt = mybir.dt

    xt2 = nc.dram_tensor("xt2", [2 * F, BL], dt.bfloat16,
                         kind="ExternalInput").ap()
    # w1b1s rows: 2j + r  (j feature-in-quad, r 0=w1 / 1=b1), cols q*H + h
    w1b1s = nc.dram_tensor("w1b1s", [8, F * H], dt.bfloat16,
                           kind="ExternalInput").ap()
    w2s = nc.dram_tensor("w2s", [H, F * E], dt.bfloat16,
                         kind="ExternalInput").ap()
    b2qs = nc.dram_tensor("b2qs", [128, NQUAD], dt.float32,
                          kind="ExternalInput").ap()
    outT = nc.dram_tensor("outT", [F * E, BL], dt.bfloat16,
                          kind="ExternalOutput").ap()

    # xt2 rows: 8q + 2j + r  (q quad, j feature-in-quad, r 0=x / 1=ones)
    xt2_r = xt2.rearrange("(q j r) n -> r j q n", j=4, r=2)  # [2,4,NQUAD,BL]

    for _ in range(CONFIG["VARIANT_ID"]):
        nc.sync.nop()

    out_pat = CONFIG["OUT_PATTERN"]
    relu_pat = CONFIG["RELU_PATTERN"]

    with tile.TileContext(nc) as tc:
        with (
            tc.tile_pool(name="params", bufs=1) as params,
            tc.tile_pool(name="xq", bufs=2) as xq_pool,
            tc.tile_pool(name="h", bufs=4) as h_pool,
            tc.tile_pool(name="outq", bufs=4) as outq_pool,
            tc.tile_pool(name="pre", bufs=3, space="PSUM") as pre_pool,
            tc.tile_pool(name="pout", bufs=2, space="PSUM") as pout_pool,
        ):
            # Startup: spread loads over engine queues so their fixed DMA
            # latencies overlap instead of serializing on one ring.
            w1b1q_sb = params.tile([128, F * H], dt.bfloat16, tag="w1b1q")
            b2_sb = params.tile([128, NQUAD], dt.float32, tag="b2qs")
            w2_sb = params.tile([H, F * E], dt.bfloat16, tag="w2s")
            warm = params.tile([128, 2], dt.float32, tag="warm")

            def load_params():
                nc.sync.dma_start(out=b2_sb[:], in_=b2qs[:])
                # split by r: each DMA hits partitions {0,32,64,96} -> 4
                # SDMA engines instead of 1
                w1b1_v = w1b1q_sb[:].rearrange("(j u) m -> u j m", u=32)
                w1b1s_v = w1b1s.rearrange("(j r) m -> r j m", r=2)
                for r in range(2):
                    nc.sync.dma_start(out=w1b1_v[r], in_=w1b1s_v[r])
                nc.gpsimd.dma_start(out=w2_sb[:], in_=w2s[:])
                # Pre-warm ACT function tables during the startup DMA wait.
                nc.scalar.activation(warm[:, 0:1], b2_sb[:, 0:1],
                                     mybir.ActivationFunctionType.Relu)
                nc.scalar.activation(warm[:, 1:2], b2_sb[:, 0:1],
                                     mybir.ActivationFunctionType.Identity)

            NIT = NCHUNK * NQUAD
            LOOK = CONFIG["LOOKAHEAD"]
            xq_tiles = {}
            pre_tiles = {}

            def load_xq(c, split_first=False):
                # xq[32j + r, 512q + cc] = xt2[8q + 2j + r, 512c + cc]
                # DMAs split by r: each spans partitions {r,32+r,64+r,96+r}
                # so the transfer spreads over SDMA engines, on the scalar
                # ring (keeps the sync ring free for out-stores). For the
                # first chunk, a small priority DMA covers quads 0-7 so the
                # pipeline can start before the bulk transfer lands.
                xq = xq_pool.tile([128, NQUAD * CHUNK], dt.bfloat16, tag="xq")
                xq_v = xq[:].rearrange("(j u) (q n) -> u j q n",
                                       u=32, n=CHUNK)
                qsplits = [(0, 8), (8, NQUAD - 8)] if split_first \
                    else [(0, NQUAD)]
                for q0, qn in qsplits:
                    for r in range(2):
                        nc.scalar.dma_start(
                            out=xq_v[r, :, q0:q0 + qn, :],
                            in_=xt2_r[r, :, q0:q0 + qn, bass.ts(c, CHUNK)],
                        )
                xq_tiles[c] = xq

            def issue_l1(it):
                # ---- L1: 4 features, row-tiled, K=2 matmuls ----
                c, q = divmod(it, NQUAD)
                xq = xq_tiles[c]
                pre_a = pre_pool.tile([128, 2 * CHUNK], dt.float32, tag="pre")
                pre_b = pre_pool.tile([128, 2 * CHUNK], dt.float32, tag="pre")
                for j in range(4):
                    tgt = pre_a if j < 2 else pre_b
                    nc.tensor.matmul(
                        tgt[:, bass.ts(j % 2, CHUNK)],
                        lhsT=w1b1q_sb[32 * j:32 * j + 2, bass.ts(q, H)],
                        rhs=xq[32 * j:32 * j + 2, bass.ts(q, CHUNK)],
                        start=True, stop=True,
                        tile_position=(32 * j, 0),
                    )
                pre_tiles[it] = (pre_a, pre_b)

            def flush_out(pend):
                pout, it = pend
                q = it % NQUAD
                outq = outq_pool.tile([128, CHUNK], dt.bfloat16, tag="outq")
                eng = out_pat[it % len(out_pat)]
                if eng == "A":
                    nc.scalar.activation(
                        outq[:], pout[:],
                        mybir.ActivationFunctionType.Identity,
                        bias=b2_sb[:, q:q + 1],
                    )
                else:
                    nc.vector.tensor_scalar_add(
                        outq[:], pout[:], b2_sb[:, q:q + 1])
                c = it // NQUAD
                # Ring follows the producing engine: the trigger then never
                # waits cross-engine (which would block that queue), and the
                # per-store descriptor generation (128 descs) splits across
                # both HWDGE rings.
                ring = nc.scalar if eng == "A" else nc.sync
                ring.dma_start(
                    out=outT[bass.ts(q, 128), bass.ts(c, CHUNK)],
                    in_=outq[:],
                )

            pending = None   # (pout_tile, it_idx) awaiting +b2/store
            load_xq(0, split_first=True)  # first so nothing delays it
            load_params()
            for it in range(LOOK):
                issue_l1(it)

            for it in range(NIT):
                c, q = divmod(it, NQUAD)
                # prefetch next chunk's x mid-way through this chunk
                if q == 8 and c + 1 < NCHUNK:
                    load_xq(c + 1)
                # L1 runs LOOK quads ahead of the relu/L2 stage
                if it + LOOK < NIT:
                    issue_l1(it + LOOK)

                pre_a, pre_b = pre_tiles.pop(it)
                # ---- relu + cast bf16, split ACT / DVE ----
                hT = h_pool.tile([128, 4 * CHUNK], dt.bfloat16, tag="h")
                for half, hsrc in ((0, pre_a), (1, pre_b)):
                    dst = hT[:, bass.ts(half, 2 * CHUNK)]
                    if relu_pat[half % len(relu_pat)] == "A":
                        nc.scalar.activation(
                            dst, hsrc[:],
                            mybir.ActivationFunctionType.Relu)
                    else:
                        nc.vector.tensor_scalar_max(dst, hsrc[:], 0.0)

                # ---- L2: 4 features col-tiled into one PSUM bank ----
                pout = pout_pool.tile([128, CHUNK], dt.float32, tag="pout")
                for j in range(4):
                    f = 4 * q + j
                    nc.tensor.matmul(
                        pout[32 * j:32 * j + 32, :],
                        lhsT=w2_sb[:, bass.ts(f, E)],
                        rhs=hT[:, bass.ts(j, CHUNK)],
                        start=True, stop=True,
                        tile_position=(0, 32 * j),
                    )

                # ---- previous quad's +b2 / cast / store ----
                if pending is not None:
                    flush_out(pending)
                pending = (pout, it)

            flush_out(pending)

    nc.compile()
    return nc


def _prep_inputs(x, w1, b1, w2, b2):
    """Host-side packing of parameters + per-core x shards."""
    w1b1s = np.zeros((8, F * H), dtype=BF16)
    for f in range(F):
        q, j = divmod(f, 4)
        w1b1s[2 * j + 0, H * q:H * q + H] = w1[f].astype(BF16)
        w1b1s[2 * j + 1, H * q:H * q + H] = b1[f].astype(BF16)

    w2s = np.ascontiguousarray(
        w2.transpose(1, 0, 2).reshape(H, F * E)).astype(BF16)
    # b2qs[32j + e, q] = b2[4q + j, e]
    b2qs = np.ascontiguousarray(
        b2.reshape(NQUAD, 4, E).transpose(1, 2, 0).reshape(128, NQUAD)
    ).astype(np.float32)

    in_maps = []
    for core in range(NCORES):
        xs = x[core * BL:(core + 1) * BL]          # [BL, F]
        xt2 = np.empty((2 * F, BL), dtype=BF16)
        xt2[0::2] = xs.T.astype(BF16)
        xt2[1::2] = BF16(1.0)
        in_maps.append({
            "xt2": xt2, "w1b1s": w1b1s, "w2s": w2s, "b2qs": b2qs,
        })
    return in_maps


def _get_compiled():
    global _COMPILED
    if _COMPILED is None:
        _COMPILED = _build_bass()
    return _COMPILED


def reset_compiled():
    global _COMPILED
    _COMPILED = None


def kernel(x, w1, b1, w2, b2, _trace=False, _trace_kwargs=None):
    nc = _get_compiled()
    in_maps = _prep_inputs(
        np.asarray(x, dtype=np.float32), np.asarray(w1, dtype=np.float32),
        np.asarray(b1, dtype=np.float32), np.asarray(w2, dtype=np.float32),
        np.asarray(b2, dtype=np.float32))
    res = run_bass_kernel_spmd(
        nc, in_maps, core_ids=list(range(NCORES)),
        trace=_trace, **(_trace_kwargs or {}))
    full = np.empty((B, F * E), dtype=np.float32)
    for i in range(NCORES):
        shard = np.asarray(res.results[i]["outT"])   # [FE, BL] bf16
        full[i * BL:(i + 1) * BL] = shard.T.astype(np.float32)
    if _trace:
        return full, res
    return full


if __name__ == "__main__":
    rng = np.random.default_rng(0)
    x = rng.standard_normal((B, F), dtype=np.float32)
    w1 = rng.standard_normal((F, H), dtype=np.float32)
    b1 = rng.standard_normal((F, H), dtype=np.float32)
    w2 = (rng.standard_normal((F, H, E), dtype=np.float32) / np.sqrt(H)).astype(np.float32)
    b2 = rng.standard_normal((F, E), dtype=np.float32) / np.sqrt(H)
    got = kernel(x=x, w1=w1, b1=b1, w2=w2, b2=b2)
    h = np.maximum(x[:, :, None] * w1[None] + b1[None], 0.0)
    want = (np.einsum("bfh,fhe->bfe", h, w2) + b2[None]).reshape(B, F * E)
    err = np.abs(got - want).max() / np.abs(want).max()
    print("self-test scale-relative max err:", err)


# revision 37
# speedup vs baseline: 1.2340x; 1.0449x over previous
"""
Trainium2 Bass kernel for nn_DenseFeatureNumericEmbedding.

Computes, per feature f (F=128 independent tiny MLPs):
    h[b,f,:]   = relu(x[b,f] * w1[f,:] + b1[f,:])            # [B, F, H]
    out[b,f,:] = h[b,f,:] @ w2[f,:,:] + b2[f,:]              # [B, F, E]
    returns out.reshape(B, F*E)                              # [16384, 4096] fp32

Sharding: data-parallel over batch across 8 NeuronCores (2048 rows/core),
params replicated. No collectives; host transposes + concatenates shards.

Per-core dataflow (per 512-batch chunk, per quad of 4 features):
  L1   TensorE: 4 row-tiled K=2 matmuls (stationary [w1[f]; b1[f]],
       moving [xT[f]; ones]) -> pre [H=128, 512] per feature in PSUM.
  RELU ScalarE activation(Relu) on features 0,1; VectorE
       tensor_scalar_max(0) on features 2,3. PSUM fp32 -> SBUF bf16 hT.
  L2   TensorE: 4 col-tiled K=128 matmuls (stationary w2[f] [H,E=32]),
       4 features packed into one PSUM bank -> poutT [FE=128, 512].
  OUT  +b2 and cast to bf16, PSUM -> SBUF, on ScalarE (Identity+bias)
       or VectorE (tensor_scalar add) per a balance pattern.
  DMA  store outT [FE, BL] bf16 to DRAM (1KB contiguous runs).
Host: transpose [FE, BL] -> [BL, FE] per shard, upcast to fp32, concat.
No PE transposes, no output-stage copies: ACT/DVE do only the two
mandatory PSUM crossings (relu 33.5M + out 8.4M elems per core).
"""

import sys

sys.path.insert(0, "/opt/trn_rl_repo")

import numpy as np
import ml_dtypes

import concourse.bass as bass
import concourse.tile as tile
from concourse import bacc, mybir
from concourse.bass_utils import run_bass_kernel_spmd

BF16 = ml_dtypes.bfloat16

B = 16384
F = 128
H = 128
E = 32
NCORES = 8
BL = B // NCORES          # 2048 rows per core
CHUNK = 512               # batch columns per inner tile (1 PSUM bank fp32)
NCHUNK = BL // CHUNK      # 4
NQUAD = F // 4            # 32 quads of 4 features

CONFIG = {
    # out-pass engine per quad index (cycled): 'A' ScalarE, 'D' VectorE
    "OUT_PATTERN": "ADADA",
    # relu engine for pre_a/pre_b: 'AD' = ACT does features 0,1; DVE 2,3
    "RELU_PATTERN": "AD",
    "LOOKAHEAD": 1,       # quads of L1 prefetch ahead of the relu/L2 stage
    "VARIANT_ID": 9,      # busts the NEFF cache between variants
}

_COMPILED = None


def _build_bass():
    nc = bacc.Bacc("TRN2", target_bir_lowering=False, debug=False,
                   num_devices=NCORES)
    dt = mybir.dt

    xt2 = nc.dram_tensor("xt2", [2 * F, BL], dt.bfloat16,
                         kind="ExternalInput").ap()
    # w1b1s rows: 2j + r  (j feature-in-quad, r 0=w1 / 1=b1), cols q*H + h
    w1b1s = nc.dram_tensor("w1b1s", [8, F * H], dt.bfloat16,
                           kind="ExternalInput").ap()
    w2s = nc.dram_tensor("w2s", [H, F * E], dt.bfloat16,
                         kind="ExternalInput").ap()
    b2qs = nc.dram_tensor("b2qs", [128, NQUAD], dt.float32,
                          kind="ExternalInput").ap()
    outT = nc.dram_tensor("outT", [F * E, BL], dt.bfloat16,
                          kind="ExternalOutput").ap()

    # xt2 rows: 8q + 2j + r  (q quad, j feature-in-quad, r 0=x / 1=ones)
    xt2_r = xt2.rearrange("(q j r) n -> r j q n", j=4, r=2)  # [2,4,NQUAD,BL]

    for _ in range(CONFIG["VARIANT_ID"]):
        nc.sync.nop()

    out_pat = CONFIG["OUT_PATTERN"]
    relu_pat = CONFIG["RELU_PATTERN"]

    with tile.TileContext(nc) as tc:
        with (
            tc.tile_pool(name="params", bufs=1) as params,
            tc.tile_pool(name="xq", bufs=2) as xq_pool,
            tc.tile_pool(name="h", bufs=4) as h_pool,
            tc.tile_pool(name="outq", bufs=4) as outq_pool,
            tc.tile_pool(name="pre", bufs=3, space="PSUM") as pre_pool,
            tc.tile_pool(name="pout", bufs=2, space="PSUM") as pout_pool,
        ):
            # Startup: spread loads over engine queues so their fixed DMA
            # latencies overlap instead of serializing on one ring.
            w1b1q_sb = params.tile([128, F * H], dt.bfloat16, tag="w1b1q")
            b2_sb = params.tile([128, NQUAD], dt.float32, tag="b2qs")
            w2_sb = params.tile([H, F * E], dt.bfloat16, tag="w2s")
            warm = params.tile([128, 2], dt.float32, tag="warm")

            def load_params():
                # split by r: each DMA hits partitions {0,32,64,96} -> 4
                # SDMA engines instead of 1; b2 last (its 128 descriptors
                # would delay w1b1's descriptor generation)
                w1b1_v = w1b1q_sb[:].rearrange("(j u) m -> u j m", u=32)
                w1b1s_v = w1b1s.rearrange("(j r) m -> r j m", r=2)
                for r in range(2):
                    nc.sync.dma_start(out=w1b1_v[r], in_=w1b1s_v[r])
                nc.sync.dma_start(out=b2_sb[:], in_=b2qs[:])
                nc.gpsimd.dma_start(out=w2_sb[:], in_=w2s[:])
                # Pre-warm ACT function tables during the startup DMA wait.
                nc.scalar.activation(warm[:, 0:1], b2_sb[:, 0:1],
                                     mybir.ActivationFunctionType.Relu)
                nc.scalar.activation(warm[:, 1:2], b2_sb[:, 0:1],
                                     mybir.ActivationFunctionType.Identity)

            NIT = NCHUNK * NQUAD
            LOOK = CONFIG["LOOKAHEAD"]
            xq_tiles = {}
            pre_tiles = {}

            def load_xq(c, split_first=False):
                # xq[32j + r, 512q + cc] = xt2[8q + 2j + r, 512c + cc]
                # DMAs split by r: each spans partitions {r,32+r,64+r,96+r}
                # so the transfer spreads over SDMA engines, on the scalar
                # ring (keeps the sync ring free for out-stores). For the
                # first chunk, a small priority DMA covers quads 0-7 so the
                # pipeline can start before the bulk transfer lands.
                xq = xq_pool.tile([128, NQUAD * CHUNK], dt.bfloat16, tag="xq")
                xq_v = xq[:].rearrange("(j u) (q n) -> u j q n",
                                       u=32, n=CHUNK)
                qsplits = [(0, 8), (8, NQUAD - 8)] if split_first \
                    else [(0, NQUAD)]
                for q0, qn in qsplits:
                    for r in range(2):
                        nc.scalar.dma_start(
                            out=xq_v[r, :, q0:q0 + qn, :],
                            in_=xt2_r[r, :, q0:q0 + qn, bass.ts(c, CHUNK)],
                        )
                xq_tiles[c] = xq

            def issue_l1(it):
                # ---- L1: 4 features, row-tiled, K=2 matmuls ----
                c, q = divmod(it, NQUAD)
                xq = xq_tiles[c]
                pre_a = pre_pool.tile([128, 2 * CHUNK], dt.float32, tag="pre")
                pre_b = pre_pool.tile([128, 2 * CHUNK], dt.float32, tag="pre")
                for j in range(4):
                    tgt = pre_a if j < 2 else pre_b
                    nc.tensor.matmul(
                        tgt[:, bass.ts(j % 2, CHUNK)],
                        lhsT=w1b1q_sb[32 * j:32 * j + 2, bass.ts(q, H)],
                        rhs=xq[32 * j:32 * j + 2, bass.ts(q, CHUNK)],
                        start=True, stop=True,
                        tile_position=(32 * j, 0),
                    )
                pre_tiles[it] = (pre_a, pre_b)

            def flush_out(pend):
                pout, it = pend
                q = it % NQUAD
                outq = outq_pool.tile([128, CHUNK], dt.bfloat16, tag="outq")
                eng = out_pat[it % len(out_pat)]
                if eng == "A":
                    nc.scalar.activation(
                        outq[:], pout[:],
                        mybir.ActivationFunctionType.Identity,
                        bias=b2_sb[:, q:q + 1],
                    )
                else:
                    nc.vector.tensor_scalar_add(
                        outq[:], pout[:], b2_sb[:, q:q + 1])
                c = it // NQUAD
                nc.sync.dma_start(
                    out=outT[bass.ts(q, 128), bass.ts(c, CHUNK)],
                    in_=outq[:],
                )

            pending = None   # (pout_tile, it_idx) awaiting +b2/store
            load_xq(0, split_first=True)  # first so nothing delays it
            load_params()
            for it in range(LOOK):
                issue_l1(it)

            for it in range(NIT):
                c, q = divmod(it, NQUAD)
                # prefetch next chunk's x mid-way through this chunk
                if q == 8 and c + 1 < NCHUNK:
                    load_xq(c + 1)
                # L1 runs LOOK quads ahead of the relu/L2 stage
                if it + LOOK < NIT:
                    issue_l1(it + LOOK)

                pre_a, pre_b = pre_tiles.pop(it)
                # ---- relu + cast bf16, split ACT / DVE ----
                hT = h_pool.tile([128, 4 * CHUNK], dt.bfloat16, tag="h")
                for half, hsrc in ((0, pre_a), (1, pre_b)):
                    dst = hT[:, bass.ts(half, 2 * CHUNK)]
                    if relu_pat[half % len(relu_pat)] == "A":
                        nc.scalar.activation(
                            dst, hsrc[:],
                            mybir.ActivationFunctionType.Relu)
                    else:
                        nc.vector.tensor_scalar_max(dst, hsrc[:], 0.0)

                # ---- L2: 4 features col-tiled into one PSUM bank ----
                pout = pout_pool.tile([128, CHUNK], dt.float32, tag="pout")
                for j in range(4):
                    f = 4 * q + j
                    nc.tensor.matmul(
                        pout[32 * j:32 * j + 32, :],
                        lhsT=w2_sb[:, bass.ts(f, E)],
                        rhs=hT[:, bass.ts(j, CHUNK)],
                        start=True, stop=True,
                        tile_position=(0, 32 * j),
                    )

                # ---- previous quad's +b2 / cast / store ----
                if pending is not None:
                    flush_out(pending)
                pending = (pout, it)

            flush_out(pending)

    nc.compile()
    return nc


def _prep_inputs(x, w1, b1, w2, b2):
    """Host-side packing of parameters + per-core x shards."""
    w1b1s = np.zeros((8, F * H), dtype=BF16)
    for f in range(F):
        q, j = divmod(f, 4)
        w1b1s[2 * j + 0, H * q:H * q + H] = w1[f].astype(BF16)
        w1b1s[2 * j + 1, H * q:H * q + H] = b1[f].astype(BF16)

    w2s = np.ascontiguousarray(
        w2.transpose(1, 0, 2).reshape(H, F * E)).astype(BF16)
    # b2qs[32j + e, q] = b2[4q + j, e]
    b2qs = np.ascontiguousarray(
        b2.reshape(NQUAD, 4, E).transpose(1, 2, 0).reshape(128, NQUAD)
    ).astype(np.float32)

    in_maps = []
    for core in range(NCORES):
        xs = x[core * BL:(core + 1) * BL]          # [BL, F]
        xt2 = np.empty((2 * F, BL), dtype=BF16)
        xt2[0::2] = xs.T.astype(BF16)
        xt2[1::2] = BF16(1.0)
        in_maps.append({
            "xt2": xt2, "w1b1s": w1b1s, "w2s": w2s, "b2qs": b2qs,
        })
    return in_maps


def _get_compiled():
    global _COMPILED
    if _COMPILED is None:
        _COMPILED = _build_bass()
    return _COMPILED


def reset_compiled():
    global _COMPILED
    _COMPILED = None


def kernel(x, w1, b1, w2, b2, _trace=False, _trace_kwargs=None):
    nc = _get_compiled()
    in_maps = _prep_inputs(
        np.asarray(x, dtype=np.float32), np.asarray(w1, dtype=np.float32),
        np.asarray(b1, dtype=np.float32), np.asarray(w2, dtype=np.float32),
        np.asarray(b2, dtype=np.float32))
    res = run_bass_kernel_spmd(
        nc, in_maps, core_ids=list(range(NCORES)),
        trace=_trace, **(_trace_kwargs or {}))
    full = np.empty((B, F * E), dtype=np.float32)
    for i in range(NCORES):
        shard = np.asarray(res.results[i]["outT"])   # [FE, BL] bf16
        full[i * BL:(i + 1) * BL] = shard.T.astype(np.float32)
    if _trace:
        return full, res
    return full


if __name__ == "__main__":
    rng = np.random.default_rng(0)
    x = rng.standard_normal((B, F), dtype=np.float32)
    w1 = rng.standard_normal((F, H), dtype=np.float32)
    b1 = rng.standard_normal((F, H), dtype=np.float32)
    w2 = (rng.standard_normal((F, H, E), dtype=np.float32) / np.sqrt(H)).astype(np.float32)
    b2 = rng.standard_normal((F, E), dtype=np.float32) / np.sqrt(H)
    got = kernel(x=x, w1=w1, b1=b1, w2=w2, b2=b2)
    h = np.maximum(x[:, :, None] * w1[None] + b1[None], 0.0)
    want = (np.einsum("bfh,fhe->bfe", h, w2) + b2[None]).reshape(B, F * E)
    err = np.abs(got - want).max() / np.abs(want).max()
    print("self-test scale-relative max err:", err)


# revision 38
# speedup vs baseline: 1.2443x; 1.0083x over previous
"""
Trainium2 Bass kernel for nn_DenseFeatureNumericEmbedding.

Computes, per feature f (F=128 independent tiny MLPs):
    h[b,f,:]   = relu(x[b,f] * w1[f,:] + b1[f,:])            # [B, F, H]
    out[b,f,:] = h[b,f,:] @ w2[f,:,:] + b2[f,:]              # [B, F, E]
    returns out.reshape(B, F*E)                              # [16384, 4096] fp32

Sharding: data-parallel over batch across 8 NeuronCores (2048 rows/core),
params replicated. No collectives; host transposes + concatenates shards.

Per-core dataflow (per 512-batch chunk, per quad of 4 features):
  L1   TensorE: 4 row-tiled K=2 matmuls (stationary [w1[f]; b1[f]],
       moving [xT[f]; ones]) -> pre [H=128, 512] per feature in PSUM.
  RELU ScalarE activation(Relu) on features 0,1; VectorE
       tensor_scalar_max(0) on features 2,3. PSUM fp32 -> SBUF bf16 hT.
  L2   TensorE: 4 col-tiled K=128 matmuls (stationary w2[f] [H,E=32]),
       4 features packed into one PSUM bank -> poutT [FE=128, 512].
  OUT  +b2 and cast to bf16, PSUM -> SBUF, on ScalarE (Identity+bias)
       or VectorE (tensor_scalar add) per a balance pattern.
  DMA  store outT [FE, BL] bf16 to DRAM (1KB contiguous runs).
Host: transpose [FE, BL] -> [BL, FE] per shard, upcast to fp32, concat.
No PE transposes, no output-stage copies: ACT/DVE do only the two
mandatory PSUM crossings (relu 33.5M + out 8.4M elems per core).
"""

import sys

sys.path.insert(0, "/opt/trn_rl_repo")

import numpy as np
import ml_dtypes

import concourse.bass as bass
import concourse.tile as tile
from concourse import bacc, mybir
from concourse.bass_utils import run_bass_kernel_spmd

BF16 = ml_dtypes.bfloat16

B = 16384
F = 128
H = 128
E = 32
NCORES = 8
BL = B // NCORES          # 2048 rows per core
CHUNK = 512               # batch columns per inner tile (1 PSUM bank fp32)
NCHUNK = BL // CHUNK      # 4
NQUAD = F // 4            # 32 quads of 4 features

CONFIG = {
    # out-pass engine per quad index (cycled): 'A' ScalarE, 'D' VectorE
    "OUT_PATTERN": "ADADA",
    # relu engine for pre_a/pre_b: 'AD' = ACT does features 0,1; DVE 2,3
    "RELU_PATTERN": "AD",
    "LOOKAHEAD": 1,       # quads of L1 prefetch ahead of the relu/L2 stage
    "VARIANT_ID": 10,      # busts the NEFF cache between variants
}

_COMPILED = None


def _build_bass():
    nc = bacc.Bacc("TRN2", target_bir_lowering=False, debug=False,
                   num_devices=NCORES)
    dt = mybir.dt

    xt2 = nc.dram_tensor("xt2", [2 * F, BL], dt.bfloat16,
                         kind="ExternalInput").ap()
    # w1b1s rows: 2j + r  (j feature-in-quad, r 0=w1 / 1=b1), cols q*H + h
    w1b1s = nc.dram_tensor("w1b1s", [8, F * H], dt.bfloat16,
                           kind="ExternalInput").ap()
    w2s = nc.dram_tensor("w2s", [H, F * E], dt.bfloat16,
                         kind="ExternalInput").ap()
    b2qs = nc.dram_tensor("b2qs", [128, NQUAD], dt.float32,
                          kind="ExternalInput").ap()
    outT = nc.dram_tensor("outT", [F * E, BL], dt.bfloat16,
                          kind="ExternalOutput").ap()

    # xt2 rows: 8q + 2j + r  (q quad, j feature-in-quad, r 0=x / 1=ones)
    xt2_r = xt2.rearrange("(q j r) n -> r j q n", j=4, r=2)  # [2,4,NQUAD,BL]

    for _ in range(CONFIG["VARIANT_ID"]):
        nc.sync.nop()

    out_pat = CONFIG["OUT_PATTERN"]
    relu_pat = CONFIG["RELU_PATTERN"]

    with tile.TileContext(nc) as tc:
        with (
            tc.tile_pool(name="params", bufs=1) as params,
            tc.tile_pool(name="xq", bufs=3) as xq_pool,
            tc.tile_pool(name="h", bufs=6) as h_pool,
            tc.tile_pool(name="outq", bufs=6) as outq_pool,
            tc.tile_pool(name="pre", bufs=3, space="PSUM") as pre_pool,
            tc.tile_pool(name="pout", bufs=2, space="PSUM") as pout_pool,
        ):
            # Startup: spread loads over engine queues so their fixed DMA
            # latencies overlap instead of serializing on one ring.
            w1b1q_sb = params.tile([128, F * H], dt.bfloat16, tag="w1b1q")
            b2_sb = params.tile([128, NQUAD], dt.float32, tag="b2qs")
            w2_sb = params.tile([H, F * E], dt.bfloat16, tag="w2s")
            warm = params.tile([128, 2], dt.float32, tag="warm")

            def load_params():
                # split by r: each DMA hits partitions {0,32,64,96} -> 4
                # SDMA engines instead of 1; b2 last (its 128 descriptors
                # would delay w1b1's descriptor generation)
                w1b1_v = w1b1q_sb[:].rearrange("(j u) m -> u j m", u=32)
                w1b1s_v = w1b1s.rearrange("(j r) m -> r j m", r=2)
                for r in range(2):
                    nc.sync.dma_start(out=w1b1_v[r], in_=w1b1s_v[r])
                nc.sync.dma_start(out=b2_sb[:], in_=b2qs[:])
                nc.gpsimd.dma_start(out=w2_sb[:], in_=w2s[:])
                # Pre-warm ACT function tables during the startup DMA wait.
                nc.scalar.activation(warm[:, 0:1], b2_sb[:, 0:1],
                                     mybir.ActivationFunctionType.Relu)
                nc.scalar.activation(warm[:, 1:2], b2_sb[:, 0:1],
                                     mybir.ActivationFunctionType.Identity)

            NIT = NCHUNK * NQUAD
            LOOK = CONFIG["LOOKAHEAD"]
            xq_tiles = {}
            pre_tiles = {}

            def load_xq(c, split_first=False):
                # xq[32j + r, 512q + cc] = xt2[8q + 2j + r, 512c + cc]
                # DMAs split by r: each spans partitions {r,32+r,64+r,96+r}
                # so the transfer spreads over SDMA engines, on the scalar
                # ring (keeps the sync ring free for out-stores). For the
                # first chunk, a small priority DMA covers quads 0-7 so the
                # pipeline can start before the bulk transfer lands.
                xq = xq_pool.tile([128, NQUAD * CHUNK], dt.bfloat16, tag="xq")
                xq_v = xq[:].rearrange("(j u) (q n) -> u j q n",
                                       u=32, n=CHUNK)
                qsplits = [(0, 2), (2, 6), (8, NQUAD - 8)] if split_first \
                    else [(0, NQUAD)]
                for q0, qn in qsplits:
                    for r in range(2):
                        nc.scalar.dma_start(
                            out=xq_v[r, :, q0:q0 + qn, :],
                            in_=xt2_r[r, :, q0:q0 + qn, bass.ts(c, CHUNK)],
                        )
                xq_tiles[c] = xq

            def issue_l1(it):
                # ---- L1: 4 features, row-tiled, K=2 matmuls ----
                c, q = divmod(it, NQUAD)
                xq = xq_tiles[c]
                pre_a = pre_pool.tile([128, 2 * CHUNK], dt.float32, tag="pre")
                pre_b = pre_pool.tile([128, 2 * CHUNK], dt.float32, tag="pre")
                for j in range(4):
                    tgt = pre_a if j < 2 else pre_b
                    nc.tensor.matmul(
                        tgt[:, bass.ts(j % 2, CHUNK)],
                        lhsT=w1b1q_sb[32 * j:32 * j + 2, bass.ts(q, H)],
                        rhs=xq[32 * j:32 * j + 2, bass.ts(q, CHUNK)],
                        start=True, stop=True,
                        tile_position=(32 * j, 0),
                    )
                pre_tiles[it] = (pre_a, pre_b)

            def flush_out(pend):
                pout, it = pend
                q = it % NQUAD
                outq = outq_pool.tile([128, CHUNK], dt.bfloat16, tag="outq")
                eng = out_pat[it % len(out_pat)]
                if eng == "A":
                    nc.scalar.activation(
                        outq[:], pout[:],
                        mybir.ActivationFunctionType.Identity,
                        bias=b2_sb[:, q:q + 1],
                    )
                else:
                    nc.vector.tensor_scalar_add(
                        outq[:], pout[:], b2_sb[:, q:q + 1])
                c = it // NQUAD
                nc.sync.dma_start(
                    out=outT[bass.ts(q, 128), bass.ts(c, CHUNK)],
                    in_=outq[:],
                )

            pending = None   # (pout_tile, it_idx) awaiting +b2/store
            load_xq(0, split_first=True)  # first so nothing delays it
            load_params()
            for it in range(LOOK):
                issue_l1(it)

            for it in range(NIT):
                c, q = divmod(it, NQUAD)
                # prefetch next chunk's x mid-way through this chunk
                if q == 8 and c + 1 < NCHUNK:
                    load_xq(c + 1)
                # L1 runs LOOK quads ahead of the relu/L2 stage
                if it + LOOK < NIT:
                    issue_l1(it + LOOK)

                pre_a, pre_b = pre_tiles.pop(it)
                # ---- relu + cast bf16, split ACT / DVE ----
                hT = h_pool.tile([128, 4 * CHUNK], dt.bfloat16, tag="h")
                for half, hsrc in ((0, pre_a), (1, pre_b)):
                    dst = hT[:, bass.ts(half, 2 * CHUNK)]
                    if relu_pat[half % len(relu_pat)] == "A":
                        nc.scalar.activation(
                            dst, hsrc[:],
                            mybir.ActivationFunctionType.Relu)
                    else:
                        nc.vector.tensor_scalar_max(dst, hsrc[:], 0.0)

                # ---- L2: 4 features col-tiled into one PSUM bank ----
                pout = pout_pool.tile([128, CHUNK], dt.float32, tag="pout")
                for j in range(4):
                    f = 4 * q + j
                    nc.tensor.matmul(
                        pout[32 * j:32 * j + 32, :],
                        lhsT=w2_sb[:, bass.ts(f, E)],
                        rhs=hT[:, bass.ts(j, CHUNK)],
                        start=True, stop=True,
                        tile_position=(0, 32 * j),
                    )

                # ---- previous quad's +b2 / cast / store ----
                if pending is not None:
                    flush_out(pending)
                pending = (pout, it)

            flush_out(pending)

    nc.compile()
    return nc


def _prep_inputs(x, w1, b1, w2, b2):
    """Host-side packing of parameters + per-core x shards."""
    w1b1s = np.zeros((8, F * H), dtype=BF16)
    for f in range(F):
        q, j = divmod(f, 4)
        w1b1s[2 * j + 0, H * q:H * q + H] = w1[f].astype(BF16)
        w1b1s[2 * j + 1, H * q:H * q + H] = b1[f].astype(BF16)

    w2s = np.ascontiguousarray(
        w2.transpose(1, 0, 2).reshape(H, F * E)).astype(BF16)
    # b2qs[32j + e, q] = b2[4q + j, e]
    b2qs = np.ascontiguousarray(
        b2.reshape(NQUAD, 4, E).transpose(1, 2, 0).reshape(128, NQUAD)
    ).astype(np.float32)

    in_maps = []
    for core in range(NCORES):
        xs = x[core * BL:(core + 1) * BL]          # [BL, F]
        xt2 = np.empty((2 * F, BL), dtype=BF16)
        xt2[0::2] = xs.T.astype(BF16)
        xt2[1::2] = BF16(1.0)
        in_maps.append({
            "xt2": xt2, "w1b1s": w1b1s, "w2s": w2s, "b2qs": b2qs,
        })
    return in_maps


def _get_compiled():
    global _COMPILED
    if _COMPILED is None:
        _COMPILED = _build_bass()
    return _COMPILED


def reset_compiled():
    global _COMPILED
    _COMPILED = None


def kernel(x, w1, b1, w2, b2, _trace=False, _trace_kwargs=None):
    nc = _get_compiled()
    in_maps = _prep_inputs(
        np.asarray(x, dtype=np.float32), np.asarray(w1, dtype=np.float32),
        np.asarray(b1, dtype=np.float32), np.asarray(w2, dtype=np.float32),
        np.asarray(b2, dtype=np.float32))
    res = run_bass_kernel_spmd(
        nc, in_maps, core_ids=list(range(NCORES)),
        trace=_trace, **(_trace_kwargs or {}))
    full = np.empty((B, F * E), dtype=np.float32)
    for i in range(NCORES):
        shard = np.asarray(res.results[i]["outT"])   # [FE, BL] bf16
        full[i * BL:(i + 1) * BL] = shard.T.astype(np.float32)
    if _trace:
        return full, res
    return full


if __name__ == "__main__":
    rng = np.random.default_rng(0)
    x = rng.standard_normal((B, F), dtype=np.float32)
    w1 = rng.standard_normal((F, H), dtype=np.float32)
    b1 = rng.standard_normal((F, H), dtype=np.float32)
    w2 = (rng.standard_normal((F, H, E), dtype=np.float32) / np.sqrt(H)).astype(np.float32)
    b2 = rng.standard_normal((F, E), dtype=np.float32) / np.sqrt(H)
    got = kernel(x=x, w1=w1, b1=b1, w2=w2, b2=b2)
    h = np.maximum(x[:, :, None] * w1[None] + b1[None], 0.0)
    want = (np.einsum("bfh,fhe->bfe", h, w2) + b2[None]).reshape(B, F * E)
    err = np.abs(got - want).max() / np.abs(want).max()
    print("self-test scale-relative max err:", err)
